# revision 2
# baseline (speedup 1.0000x reference)
"""Trainium2 Bass kernel for nn_AlignerOT, v2.

Math identical to the validated baseline: per-sample log-domain Sinkhorn
reproduced by kernel-space iterations u <- 1/(M v), v <- 1/(M^T u) on
M = exp(pre + BR), restabilized by absorbing ln u, ln v into row/col biases
BR, BC at segment boundaries.  50 iterations total.

v2 performance structure:
- 3 segments (17/17/16) instead of 5x10 (f32 range validated: umax ~1e29 <<
  3.4e38).
- cohorts of 4 samples on PE col groups 0/32/64/96; matvec matmuls
  interleaved kc-outer x h x sample => 8 concurrent accumulation chains
  (HW-measured 74ns/MM effective vs ~780ns for a single serialized chain).
- pre tiles built by ONE K=5-packed fp16 matmul each.  Row banks per sample
  ([37, D] fp16; matmul operand base_partition must be 0/32/64, and tile
  memory cost is free-size only, so two 5-row windows at partitions 0 and
  32 are free):
    ybank rows 0:5  = (-1, -1, yh, yh, yl)    M-pre lhsT
    ybank rows 32:37 = (BRh, BRl, yh, yh, yl) MT-pre rhs
    sbank rows 0:5  = (rth, rtl, sh, sl, sh)  M-pre rhs
    sbank rows 32:37 = (1, 1, sh, sl, sh)     MT-pre lhsT
  (engine writes must start at partition 0/32/64/96, so the rebuilt-per-
  segment rows rth/rtl and BRh/BRl sit at window starts)
  M-pre  = 2g*Y_j*s_k - (g*s_k^2 - BC_k);  MT-pre = 2g*s_k*Y_j + BR_j
- MT built hybrid: `hybrid_kc` of 8 kc-rows by direct K=5 transposed-pre
  build with exp bias (BC_k - g*s_k^2); the rest by PE transpose of M tiles.
- per-side chain: strided [4,512] psum->SBUF copies (act h0 + DVE h1 in
  parallel), K=4 batched transpose-trick matmuls (4 samples at once), one
  DVE reciprocal.
Sharding: data-parallel over N (16 samples/core); AllReduce sum_i P[i]*D;
    ot = (SCALE/N)*AR + delta; out rows per-core; host concat.
"""

import numpy as np

N_CORES = 8
N_GLOB = 128
NS = N_GLOB // N_CORES   # 16
S_IN = 768
SC = S_IN // 128         # 6
D = 1024
JC = D // 128            # 8
EPS = 0.1
SCALE = 300.0
GAMMA = SCALE / EPS
RT2G = float(np.sqrt(2.0 * GAMMA))
RTG = float(np.sqrt(GAMMA))
SEG_LENS = (17, 17, 16)
GROUP = 4

_cache = {}


def build(seg_lens=SEG_LENS, ns=NS, group=GROUP, n_cores=N_CORES,
          skip_collective=False, hybrid_kc=3):
    import concourse.bass as bass
    import concourse.bacc as bacc
    import concourse.tile as tile
    import concourse.mybir as mybir
    from concourse.masks import make_identity

    fp32 = mybir.dt.float32
    bf16 = mybir.dt.bfloat16
    fp16 = mybir.dt.float16
    AF = mybir.ActivationFunctionType
    ALU = mybir.AluOpType
    AX = mybir.AxisListType
    ET = mybir.EngineType

    nc = bacc.Bacc("TRN2", target_bir_lowering=False, debug=False,
                   num_devices=n_cores)

    x_d = nc.dram_tensor("x", [ns, S_IN], fp32, kind="ExternalInput")
    y_d = nc.dram_tensor("y", [ns, D], fp32, kind="ExternalInput")
    w_d = nc.dram_tensor("w", [D, S_IN], fp32, kind="ExternalInput")
    b_d = nc.dram_tensor("bvec", [1, D], fp32, kind="ExternalInput")
    delta_d = nc.dram_tensor("delta", [D, D], fp32, kind="ExternalInput")
    out_d = nc.dram_tensor("out", [ns, D], fp32, kind="ExternalOutput")

    G = group
    assert ns % G == 0

    with tile.TileContext(nc) as tc:
        with (
            tc.tile_pool(name="const", bufs=1) as cpool,
            tc.tile_pool(name="rdata", bufs=1) as rpool,
            tc.tile_pool(name="acc", bufs=1) as apool,
            tc.tile_pool(name="dram", bufs=2, space="DRAM") as dpool,
        ):
            identh = cpool.tile([128, 128], fp16)
            make_identity(nc, identh[:])
            identb = cpool.tile([128, 128], bf16)
            make_identity(nc, identb[:])

            # ---------------- phase 1: src = X @ W.T + b --------------------
            src_sb = rpool.tile([ns, D], fp32)
            y_sb = rpool.tile([ns, D], fp32)
            nc.sync.dma_start(y_sb[:], y_d.ap()[:])
            srccol = rpool.tile([128, ns, JC], fp32)
            with (
                tc.tile_pool(name="wls", bufs=1) as wpool,
                tc.tile_pool(name="ph1ps", bufs=2, space="PSUM") as ph1ps,
            ):
                identf = wpool.tile([128, 128], fp32)
                make_identity(nc, identf[:])
                xt = wpool.tile([128, SC, ns], fp32)
                for sc in range(SC):
                    nc.sync.dma_start(
                        xt[:, sc, :],
                        x_d.ap()[:, sc * 128:(sc + 1) * 128].rearrange(
                            "n p -> p n"))
                ones16 = wpool.tile([1, ns], fp32)
                nc.gpsimd.memset(ones16[:], 1.0)
                b_row = wpool.tile([1, D], fp32)
                nc.sync.dma_start(b_row[:], b_d.ap()[:])
                w_sb = wpool.tile([128, JC, S_IN], fp32)
                nc.sync.dma_start(
                    w_sb[:], w_d.ap().rearrange("(dc p) s -> p dc s", p=128))
                wt = wpool.tile([128, SC, D], fp32)
                for dc in range(JC):
                    for sc in range(SC):
                        pst = ph1ps.tile([128, 512], fp32, tag="p1")
                        nc.tensor.transpose(
                            pst[:, :128],
                            w_sb[:, dc, sc * 128:(sc + 1) * 128], identf[:])
                        nc.vector.tensor_copy(
                            wt[:, sc, dc * 128:(dc + 1) * 128], pst[:, :128])
                for h in range(2):
                    ps_src = ph1ps.tile([128, 512], fp32, tag="p1")
                    for sc in range(SC):
                        nc.tensor.matmul(
                            ps_src[:ns], xt[:, sc, :],
                            wt[:, sc, h * 512:(h + 1) * 512],
                            start=(sc == 0), stop=False)
                    nc.tensor.matmul(
                        ps_src[:ns], ones16[:],
                        b_row[:, h * 512:(h + 1) * 512],
                        start=False, stop=True)
                    nc.scalar.activation(
                        src_sb[:, h * 512:(h + 1) * 512], ps_src[:ns],
                        AF.Copy)
                for c in range(JC):
                    pst = ph1ps.tile([128, 512], fp32, tag="p1")
                    nc.tensor.transpose(
                        pst[:, :ns], src_sb[:, c * 128:(c + 1) * 128],
                        identf[:ns, :ns])
                    nc.vector.tensor_copy(srccol[:, :, c], pst[:, :ns])

            pacc = apool.tile([128, JC, D], fp32)
            nc.gpsimd.memset(pacc[:], 0.0)

            # ------------- phase 2: per-sample Sinkhorn ---------------------
            with (
                tc.tile_pool(name="mats", bufs=1) as mpool,
                tc.tile_pool(name="rows", bufs=1) as wrow,
                tc.tile_pool(name="vecs", bufs=1) as vpool,
                tc.tile_pool(name="ps_mv", bufs=1, space="PSUM") as ps_mv,
                tc.tile_pool(name="ps_sm", bufs=2, space="PSUM") as ps_sm,
                tc.tile_pool(name="ps_tr", bufs=2, space="PSUM") as ps_tr,
            ):
                # cohort-shared vector state, flat col index = jc*G + s
                NV = JC * G
                BR = vpool.tile([128, NV], fp32, tag="BR")
                BC = vpool.tile([128, NV], fp32, tag="BC")
                S2 = vpool.tile([128, NV], fp32, tag="S2")
                AUX = vpool.tile([128, NV], fp32, tag="AUX")
                NAUX = vpool.tile([128, NV], fp32, tag="NAUX")
                SCR = vpool.tile([128, NV], fp32, tag="SCR")
                LNT = vpool.tile([128, NV], fp32, tag="LNT")
                UF = vpool.tile([128, NV], fp32, tag="UF")
                VF = vpool.tile([128, NV], fp32, tag="VF")
                MSK = vpool.tile([128, NV], mybir.dt.uint8, tag="MSK")
                svb_u = vpool.tile([128, NV], bf16, tag="svbu")
                svb_v = vpool.tile([128, NV], bf16, tag="svbv")
                svhp = vpool.tile([128, JC, 2, G], fp16, tag="svhp")
                svbr = vpool.tile([128, JC, 2, G], fp16, tag="svbr")
                F0 = vpool.tile([128, 512], bf16, tag="F0")
                F1 = vpool.tile([128, 512], bf16, tag="F1")
                # two persistent [128,2,512] psum tiles, manually ping-
                # ponged for matvec / pre-tile use.  Persistent (not pool-
                # rotated) because matvec matmuls write only partitions
                # {32s}; full-tile reads of rotated tiles trip the race
                # detector on the never-written rows.  memset once below.
                pssA = ps_mv.tile([128, 2, 512], fp32, tag="pssA")
                pssB = ps_mv.tile([128, 2, 512], fp32, tag="pssB")
                nc.vector.memset(pssA[:], 0.0)
                nc.vector.memset(pssB[:], 0.0)
                mv_state = [0]

                def mv_tile():
                    t = pssA if mv_state[0] == 0 else pssB
                    mv_state[0] ^= 1
                    return t
                stg16 = vpool.tile([G, D], fp16, tag="stg16")
                stg32 = vpool.tile([G, D], fp32, tag="stg32")
                stgA = vpool.tile([G, D], fp32, tag="stgA")

                Ms, MTs, ybanks, sbanks = [], [], [], []
                for s in range(G):
                    Ms.append(mpool.tile([128, JC, D], bf16, tag=f"M{s}",
                                         name=f"M{s}"))
                    MTs.append(mpool.tile([128, JC, D], bf16, tag=f"MT{s}",
                                          name=f"MT{s}"))
                    ybanks.append(wrow.tile([37, D], fp16, tag=f"yb{s}",
                                            name=f"yb{s}"))
                    sbanks.append(wrow.tile([37, D], fp16, tag=f"sb{s}",
                                            name=f"sb{s}"))

                def col(jc, s):
                    return slice(jc * G + s, jc * G + s + 1)

                def scol(s):
                    return slice(s, NV, G)

                def setup_splits(c0, srcrow, hi_rows, lo_rows, banks):
                    """fp16 hi/lo split of RT2G*srcrow[c0:c0+G] -> per-
                    sample bank rows via DMA (staged at partition 0 since
                    engine ops need 32-aligned partition starts)."""
                    nc.sync.dma_start(stgA[:], srcrow[c0:c0 + G, :])
                    nc.vector.tensor_scalar_mul(stgA[:], stgA[:], RT2G)
                    nc.vector.tensor_copy(stg16[:], stgA[:])         # hi
                    nc.vector.tensor_copy(stg32[:], stg16[:])
                    nc.vector.tensor_sub(stg32[:], stgA[:], stg32[:])
                    for s in range(G):
                        for r in hi_rows:
                            nc.sync.dma_start(banks[s][r:r + 1, :],
                                              stg16[s:s + 1, :])
                    nc.vector.tensor_copy(stg16[:], stg32[:])        # lo
                    for s in range(G):
                        for r in lo_rows:
                            nc.sync.dma_start(banks[s][r:r + 1, :],
                                              stg16[s:s + 1, :])

                def row_from_cols(svt, s, dst_bank, r0):
                    """transpose-trick: fp16 hi/lo col pairs svt[:, cc, :, s]
                    -> rows r0, r0+1 of dst_bank via K=2 matmuls + copies."""
                    for h in range(2):
                        psr = ps_sm.tile([128, 512], fp32, tag="sm")
                        for c in range(4):
                            cc = h * 4 + c
                            nc.tensor.matmul(
                                psr[0:2, c * 128:(c + 1) * 128],
                                svt[:, cc, :, s], identh[:],
                                start=True, stop=True)
                        if h == 0:
                            nc.scalar.activation(
                                dst_bank[r0:r0 + 2, 0:512],
                                psr[0:2, :], AF.Copy)
                        else:
                            nc.vector.tensor_copy(
                                dst_bank[r0:r0 + 2, 512:1024],
                                psr[0:2, :])

                def hilo_cols(src_f32, s, dst):
                    """fp16 hi/lo split of src_f32[:, scol(s)] into
                    dst[:, :, 0:2, s]"""
                    nc.vector.tensor_copy(dst[:, :, 0, s], src_f32[:, scol(s)])
                    nc.vector.tensor_copy(SCR[:, scol(s)], dst[:, :, 0, s])
                    nc.vector.tensor_sub(SCR[:, scol(s)], src_f32[:, scol(s)],
                                         SCR[:, scol(s)])
                    nc.vector.tensor_copy(dst[:, :, 1, s], SCR[:, scol(s)])

                def rowterm_update():
                    """sbank rows 0,1 <- fp16 hi/lo of (S2-BC) per sample"""
                    nc.vector.tensor_sub(AUX[:], S2[:], BC[:])
                    for s in range(G):
                        hilo_cols(AUX, s, svhp)
                        row_from_cols(svhp, s, sbanks[s], 0)

                def brrow_update():
                    """ybank rows 32,33 <- fp16 hi/lo of BR per sample;
                    also NAUX = BC - S2 (direct-MT exp bias)."""
                    nc.vector.tensor_sub(NAUX[:], BC[:], S2[:])
                    for s in range(G):
                        hilo_cols(BR, s, svbr)
                        row_from_cols(svbr, s, ybanks[s], 32)

                def pre_mm(ps, s, jc, h):
                    nc.tensor.matmul(
                        ps[:, h, :], ybanks[s][0:5, jc * 128:(jc + 1) * 128],
                        sbanks[s][0:5, h * 512:(h + 1) * 512],
                        start=True, stop=True)

                def preT_mm(ps, s, kc, h):
                    nc.tensor.matmul(
                        ps[:, h, :],
                        sbanks[s][32:37, kc * 128:(kc + 1) * 128],
                        ybanks[s][32:37, h * 512:(h + 1) * 512],
                        start=True, stop=True)

                def mbuild():
                    """M = exp(pre + BR); MT hybrid direct/transpose."""
                    for s in range(G):
                        for jc in range(JC):
                            ps = mv_tile()
                            pre_mm(ps, s, jc, 0)
                            pre_mm(ps, s, jc, 1)
                            nc.scalar.activation(
                                Ms[s][:, jc, :], ps[:],
                                AF.Exp, bias=BR[:, col(jc, s)])
                    for s in range(G):
                        for kc in range(JC):
                            if kc < hybrid_kc:
                                ps = mv_tile()
                                preT_mm(ps, s, kc, 0)
                                preT_mm(ps, s, kc, 1)
                                nc.scalar.activation(
                                    MTs[s][:, kc, :], ps[:],
                                    AF.Exp, bias=NAUX[:, col(kc, s)])
                            else:
                                for hj in range(2):
                                    ptr = ps_tr.tile([128, 512], bf16,
                                                     tag="tr")
                                    for q in range(4):
                                        jc = hj * 4 + q
                                        nc.tensor.transpose(
                                            ptr[:, q * 128:(q + 1) * 128],
                                            Ms[s][:, jc,
                                                  kc * 128:(kc + 1) * 128],
                                            identb[:])
                                    nc.vector.tensor_copy(
                                        MTs[s][:, kc,
                                               hj * 512:(hj + 1) * 512],
                                        ptr[:])

                def side(use_mt, invec, uf_out, svb_out):
                    """uf_out None => skip f32 copy (only needed for the
                    absorb at segment end)"""
                    """one matvec side for the whole cohort: 8 interleaved
                    accumulation chains on 4 PE col groups."""
                    pss = mv_tile()
                    for kc in range(JC):
                        for h in range(2):
                            for s in range(G):
                                mat = MTs[s] if use_mt else Ms[s]
                                nc.tensor.matmul(
                                    pss[32 * s:32 * s + 1, h, :],
                                    invec[:, col(kc, s)],
                                    mat[:, kc, h * 512:(h + 1) * 512],
                                    start=(kc == 0), stop=(kc == JC - 1),
                                    tile_position=(0, 32 * s))
                    # full-tile copies (engines need unit partition step);
                    # sample rows sit at partitions 32s inside F0/F1
                    nc.scalar.activation(F0[:], pss[:, 0, :], AF.Copy)
                    nc.vector.tensor_copy(F1[:], pss[:, 1, :])
                    # row->column: psc[:, c*G+s] = F_half[32s, c*128:...]
                    # = (F-chunk)^T @ e_{32s}; strided-identity rhs gives
                    # all 4 samples in one N=4 matmul per chunk
                    psc = ps_sm.tile([128, 512], fp32, tag="sm")
                    for c in range(4):
                        nc.tensor.matmul(
                            psc[:, (c * G):(c * G + G)],
                            F0[:, c * 128:(c + 1) * 128],
                            identb[:, 0:128:32], start=True, stop=True)
                    for c in range(4):
                        nc.tensor.matmul(
                            psc[:, ((c + 4) * G):((c + 4) * G + G)],
                            F1[:, c * 128:(c + 1) * 128],
                            identb[:, 0:128:32], start=True, stop=True)
                    half = NV // 2
                    with nc.allow_low_precision(
                            reason="recip rounds to bf16 on write; same "
                                   "values as f32-recip-then-cast"):
                        nc.vector.reciprocal(svb_out[:, 0:half],
                                             psc[:, 0:half])
                        nc.vector.reciprocal(svb_out[:, half:NV],
                                             psc[:, half:NV])
                    if uf_out is not None:
                        nc.vector.reciprocal(uf_out[:], psc[:, 0:NV])

                LN2x34 = float(34.0 * np.log(2.0))

                def absorb(vec_f32, bias):
                    # two-range ln: act-engine Ln accepts |x| <= 2^64 and
                    # normals only; u spans ~[1e-29, 1e29].  ln(u) =
                    # Ln(u*2^-34)+34ln2 for u>=1, Ln(u*2^34)-34ln2 for u<=1.
                    nc.vector.tensor_scalar_max(SCR[:], vec_f32[:], 1.0)
                    nc.scalar.activation(LNT[:], SCR[:], AF.Ln,
                                         scale=2.0 ** -34)
                    nc.vector.tensor_scalar_add(LNT[:], LNT[:], LN2x34)
                    nc.vector.tensor_scalar_min(SCR[:], vec_f32[:], 1.0)
                    nc.scalar.activation(SCR[:], SCR[:], AF.Ln,
                                         scale=2.0 ** 34)
                    nc.vector.tensor_scalar_sub(SCR[:], SCR[:], LN2x34)
                    nc.vector.tensor_scalar(MSK[:], vec_f32[:], 1.0, None,
                                            op0=ALU.is_ge)
                    nc.vector.select(SCR[:], MSK[:], LNT[:], SCR[:])
                    nc.vector.tensor_add(bias[:], bias[:], SCR[:])

                def seg_body(sl):
                    rowterm_update()
                    brrow_update()
                    mbuild()
                    nc.gpsimd.memset(svb_v[:], 1.0)
                    for t in range(sl - 1):
                        side(True, svb_v, None, svb_u)
                        side(False, svb_u, None, svb_v)
                    side(True, svb_v, UF, svb_u)
                    side(False, svb_u, VF, svb_v)
                    absorb(UF, BR)
                    absorb(VF, BC)

                for c0 in range(0, ns, G):
                    # ---- per-sample setup ----
                    setup_splits(c0, y_sb, hi_rows=(2, 3, 34, 35),
                                 lo_rows=(4, 36), banks=ybanks)
                    setup_splits(c0, src_sb, hi_rows=(2, 4, 34, 36),
                                 lo_rows=(3, 35), banks=sbanks)
                    for s in range(G):
                        nc.gpsimd.memset(ybanks[s][0:2, :], -1.0)
                        nc.gpsimd.memset(sbanks[s][32:34, :], 1.0)
                        nc.scalar.activation(S2[:, scol(s)],
                                             srccol[:, c0 + s, :],
                                             AF.Square, scale=RTG)
                    nc.gpsimd.memset(BC[:], 0.0)

                    # ---- init pass: BR = -max_k(pre with BC=0) ----
                    rowterm_update()
                    for s in range(G):
                        for jc in range(JC):
                            ps = mv_tile()
                            pre_mm(ps, s, jc, 0)
                            pre_mm(ps, s, jc, 1)
                            nc.vector.tensor_reduce(
                                out=SCR[:, col(jc, s)], in_=ps[:, 0, :],
                                op=ALU.max, axis=AX.X)
                            nc.vector.tensor_reduce(
                                out=LNT[:, col(jc, s)], in_=ps[:, 1, :],
                                op=ALU.max, axis=AX.X)
                    nc.vector.tensor_max(SCR[:], SCR[:], LNT[:])
                    nc.vector.tensor_scalar_mul(BR[:], SCR[:], -1.0)

                    # ---- segments ----
                    if seg_lens[0] == seg_lens[1] and len(seg_lens) == 3:
                        with tc.For_i(0, 2, 1, hint_engines=(ET.PE,)):
                            seg_body(seg_lens[0])
                        seg_body(seg_lens[2])
                    else:
                        for sl in seg_lens:
                            seg_body(sl)

                    # ---- P accumulation: D*P = exp(pre + BR) --------------
                    rowterm_update()
                    for s in range(G):
                        for jc in range(JC):
                            ps = mv_tile()
                            pre_mm(ps, s, jc, 0)
                            pre_mm(ps, s, jc, 1)
                            nc.scalar.activation(
                                MTs[s][:, jc, :], ps[:],
                                AF.Exp, bias=BR[:, col(jc, s)])
                    for jc in range(JC):
                        for h in range(2):
                            pa = ps_sm.tile([128, 512], fp32, tag="sm")
                            for s in range(G):
                                nc.tensor.matmul(
                                    pa[:], identb[:],
                                    MTs[s][:, jc, h * 512:(h + 1) * 512],
                                    start=(s == 0), stop=(s == G - 1))
                            nc.vector.tensor_add(
                                pacc[:, jc, h * 512:(h + 1) * 512],
                                pacc[:, jc, h * 512:(h + 1) * 512],
                                pa[:])

            # ------------- phase 3: AllReduce + finale ----------------------
            pacc_b = dpool.tile([D, D], fp32)
            pall_b = dpool.tile(
                [D, D], fp32,
                addr_space="Shared" if n_cores > 4 else "Local")
            nc.sync.dma_start(
                pacc_b[:].rearrange("(jc p) k -> p jc k", p=128), pacc[:])
            if skip_collective:
                nc.sync.dma_start(pall_b[:], pacc_b[:])
            else:
                import concourse.mybir as mybir2
                nc.gpsimd.collective_compute(
                    "AllReduce", mybir2.AluOpType.add,
                    replica_groups=[list(range(n_cores))],
                    ins=[pacc_b.opt()], outs=[pall_b.opt()],
                )
            with (
                tc.tile_pool(name="fin", bufs=1) as fpool,
                tc.tile_pool(name="ph3ps", bufs=2, space="PSUM") as ph3ps,
            ):
                ot = fpool.tile([128, JC, D], fp32)
                nc.sync.dma_start(
                    ot[:], pall_b[:].rearrange("(jc p) k -> p jc k", p=128))
                dl = fpool.tile([128, JC, D], fp32)
                nc.sync.dma_start(
                    dl[:],
                    delta_d.ap().rearrange("(jc p) k -> p jc k", p=128))
                nc.vector.tensor_scalar_mul(ot[:], ot[:], SCALE / N_GLOB)
                nc.vector.tensor_add(ot[:], ot[:], dl[:])
                out_sb = fpool.tile([ns, D], fp32)
                for h in range(2):
                    pso = ph3ps.tile([128, 512], fp32, tag="p3")
                    for jc in range(JC):
                        nc.tensor.matmul(
                            pso[:ns], srccol[:, :, jc],
                            ot[:, jc, h * 512:(h + 1) * 512],
                            start=(jc == 0), stop=(jc == JC - 1))
                    nc.scalar.activation(
                        out_sb[:, h * 512:(h + 1) * 512], pso[:ns], AF.Copy)
                nc.sync.dma_start(out_d.ap()[:], out_sb[:])

    nc.compile()
    return nc


def kernel(**inputs):
    X = np.ascontiguousarray(inputs["X"], np.float32)
    Y = np.ascontiguousarray(inputs["Y"], np.float32)
    W = np.ascontiguousarray(inputs["W"], np.float32)
    b = np.ascontiguousarray(inputs["b"], np.float32).reshape(1, D)
    delta = np.ascontiguousarray(inputs["delta_ot"], np.float32)

    from concourse import bass_utils

    if "nc" not in _cache:
        _cache["nc"] = build()
    nc = _cache["nc"]

    in_maps = []
    for c in range(N_CORES):
        sl = slice(c * NS, (c + 1) * NS)
        in_maps.append({
            "x": X[sl], "y": Y[sl], "w": W, "bvec": b, "delta": delta,
        })
    res = bass_utils.run_bass_kernel_spmd(
        nc, in_maps, core_ids=list(range(N_CORES)))
    out = np.concatenate([res.results[c]["out"] for c in range(N_CORES)],
                         axis=0)
    return out.astype(np.float32)


if __name__ == "__main__":
    import reference
    ins = reference.setup_inputs()
    ins = {k: np.asarray(v) for k, v in ins.items()}
    got = kernel(**ins)
    print("out", got.shape, got.dtype)


# revision 3
# speedup vs baseline: 1.0558x; 1.0558x over previous
"""Trainium2 Bass kernel for nn_AlignerOT, v2.

Math identical to the validated baseline: per-sample log-domain Sinkhorn
reproduced by kernel-space iterations u <- 1/(M v), v <- 1/(M^T u) on
M = exp(pre + BR), restabilized by absorbing ln u, ln v into row/col biases
BR, BC at segment boundaries.  50 iterations total.

v2 performance structure:
- 3 segments (17/17/16) instead of 5x10 (f32 range validated: umax ~1e29 <<
  3.4e38).
- cohorts of 4 samples on PE col groups 0/32/64/96; matvec matmuls
  interleaved kc-outer x h x sample => 8 concurrent accumulation chains
  (HW-measured 74ns/MM effective vs ~780ns for a single serialized chain).
- pre tiles built by ONE K=5-packed fp16 matmul each.  Row banks per sample
  ([37, D] fp16; matmul operand base_partition must be 0/32/64, and tile
  memory cost is free-size only, so two 5-row windows at partitions 0 and
  32 are free):
    ybank rows 0:5  = (-1, -1, yh, yh, yl)    M-pre lhsT
    ybank rows 32:37 = (BRh, BRl, yh, yh, yl) MT-pre rhs
    sbank rows 0:5  = (rth, rtl, sh, sl, sh)  M-pre rhs
    sbank rows 32:37 = (1, 1, sh, sl, sh)     MT-pre lhsT
  (engine writes must start at partition 0/32/64/96, so the rebuilt-per-
  segment rows rth/rtl and BRh/BRl sit at window starts)
  M-pre  = 2g*Y_j*s_k - (g*s_k^2 - BC_k);  MT-pre = 2g*s_k*Y_j + BR_j
- MT built hybrid: `hybrid_kc` of 8 kc-rows by direct K=5 transposed-pre
  build with exp bias (BC_k - g*s_k^2); the rest by PE transpose of M tiles.
- per-side chain: strided [4,512] psum->SBUF copies (act h0 + DVE h1 in
  parallel), K=4 batched transpose-trick matmuls (4 samples at once), one
  DVE reciprocal.
Sharding: data-parallel over N (16 samples/core); AllReduce sum_i P[i]*D;
    ot = (SCALE/N)*AR + delta; out rows per-core; host concat.
"""

import numpy as np

N_CORES = 8
N_GLOB = 128
NS = N_GLOB // N_CORES   # 16
S_IN = 768
SC = S_IN // 128         # 6
D = 1024
JC = D // 128            # 8
EPS = 0.1
SCALE = 300.0
GAMMA = SCALE / EPS
RT2G = float(np.sqrt(2.0 * GAMMA))
RTG = float(np.sqrt(GAMMA))
SEG_LENS = (17, 17, 16)
GROUP = 4

_cache = {}


def build(seg_lens=SEG_LENS, ns=NS, group=GROUP, n_cores=N_CORES,
          skip_collective=False, hybrid_kc=3):
    import concourse.bass as bass
    import concourse.bacc as bacc
    import concourse.tile as tile
    import concourse.mybir as mybir
    from concourse.masks import make_identity

    fp32 = mybir.dt.float32
    bf16 = mybir.dt.bfloat16
    fp16 = mybir.dt.float16
    AF = mybir.ActivationFunctionType
    ALU = mybir.AluOpType
    AX = mybir.AxisListType
    ET = mybir.EngineType

    nc = bacc.Bacc("TRN2", target_bir_lowering=False, debug=False,
                   num_devices=n_cores)

    x_d = nc.dram_tensor("x", [ns, S_IN], fp32, kind="ExternalInput")
    y_d = nc.dram_tensor("y", [ns, D], fp32, kind="ExternalInput")
    w_d = nc.dram_tensor("w", [D, S_IN], fp32, kind="ExternalInput")
    b_d = nc.dram_tensor("bvec", [1, D], fp32, kind="ExternalInput")
    delta_d = nc.dram_tensor("delta", [D, D], fp32, kind="ExternalInput")
    out_d = nc.dram_tensor("out", [ns, D], fp32, kind="ExternalOutput")

    G = group
    assert ns % G == 0

    with tile.TileContext(nc) as tc:
        with (
            tc.tile_pool(name="const", bufs=1) as cpool,
            tc.tile_pool(name="rdata", bufs=1) as rpool,
            tc.tile_pool(name="acc", bufs=1) as apool,
            tc.tile_pool(name="dram", bufs=2, space="DRAM") as dpool,
        ):
            identh = cpool.tile([128, 128], fp16)
            make_identity(nc, identh[:])
            identb = cpool.tile([128, 128], bf16)
            make_identity(nc, identb[:])

            # ---------------- phase 1: src = X @ W.T + b --------------------
            src_sb = rpool.tile([ns, D], fp32)
            y_sb = rpool.tile([ns, D], fp32)
            nc.sync.dma_start(y_sb[:], y_d.ap()[:])
            srccol = rpool.tile([128, ns, JC], fp32)
            with (
                tc.tile_pool(name="wls", bufs=1) as wpool,
                tc.tile_pool(name="ph1ps", bufs=2, space="PSUM") as ph1ps,
            ):
                identf = wpool.tile([128, 128], fp32)
                make_identity(nc, identf[:])
                xt = wpool.tile([128, SC, ns], fp32)
                for sc in range(SC):
                    nc.sync.dma_start(
                        xt[:, sc, :],
                        x_d.ap()[:, sc * 128:(sc + 1) * 128].rearrange(
                            "n p -> p n"))
                ones16 = wpool.tile([1, ns], fp32)
                nc.gpsimd.memset(ones16[:], 1.0)
                b_row = wpool.tile([1, D], fp32)
                nc.sync.dma_start(b_row[:], b_d.ap()[:])
                w_sb = wpool.tile([128, JC, S_IN], fp32)
                nc.sync.dma_start(
                    w_sb[:], w_d.ap().rearrange("(dc p) s -> p dc s", p=128))
                wt = wpool.tile([128, SC, D], fp32)
                for dc in range(JC):
                    for sc in range(SC):
                        pst = ph1ps.tile([128, 512], fp32, tag="p1")
                        nc.tensor.transpose(
                            pst[:, :128],
                            w_sb[:, dc, sc * 128:(sc + 1) * 128], identf[:])
                        nc.vector.tensor_copy(
                            wt[:, sc, dc * 128:(dc + 1) * 128], pst[:, :128])
                for h in range(2):
                    ps_src = ph1ps.tile([128, 512], fp32, tag="p1")
                    for sc in range(SC):
                        nc.tensor.matmul(
                            ps_src[:ns], xt[:, sc, :],
                            wt[:, sc, h * 512:(h + 1) * 512],
                            start=(sc == 0), stop=False)
                    nc.tensor.matmul(
                        ps_src[:ns], ones16[:],
                        b_row[:, h * 512:(h + 1) * 512],
                        start=False, stop=True)
                    nc.scalar.activation(
                        src_sb[:, h * 512:(h + 1) * 512], ps_src[:ns],
                        AF.Copy)
                for c in range(JC):
                    pst = ph1ps.tile([128, 512], fp32, tag="p1")
                    nc.tensor.transpose(
                        pst[:, :ns], src_sb[:, c * 128:(c + 1) * 128],
                        identf[:ns, :ns])
                    nc.vector.tensor_copy(srccol[:, :, c], pst[:, :ns])

            pacc = apool.tile([128, JC, D], fp32)
            nc.gpsimd.memset(pacc[:], 0.0)

            # ------------- phase 2: per-sample Sinkhorn ---------------------
            with (
                tc.tile_pool(name="mats", bufs=1) as mpool,
                tc.tile_pool(name="rows", bufs=1) as wrow,
                tc.tile_pool(name="vecs", bufs=1) as vpool,
                tc.tile_pool(name="ps_mv", bufs=1, space="PSUM") as ps_mv,
                tc.tile_pool(name="ps_sm", bufs=2, space="PSUM") as ps_sm,
                tc.tile_pool(name="ps_tr", bufs=2, space="PSUM") as ps_tr,
            ):
                # cohort-shared vector state, flat col index = jc*G + s
                NV = JC * G
                BR = vpool.tile([128, NV], fp32, tag="BR")
                BC = vpool.tile([128, NV], fp32, tag="BC")
                S2 = vpool.tile([128, NV], fp32, tag="S2")
                AUX = vpool.tile([128, NV], fp32, tag="AUX")
                NAUX = vpool.tile([128, NV], fp32, tag="NAUX")
                SCR = vpool.tile([128, NV], fp32, tag="SCR")
                LNT = vpool.tile([128, NV], fp32, tag="LNT")
                UF = vpool.tile([128, NV], fp32, tag="UF")
                VF = vpool.tile([128, NV], fp32, tag="VF")
                MSK = vpool.tile([128, NV], mybir.dt.uint8, tag="MSK")
                svb_u = vpool.tile([128, NV], bf16, tag="svbu")
                svb_v = vpool.tile([128, NV], bf16, tag="svbv")
                svhp = vpool.tile([128, JC, 2, G], fp16, tag="svhp")
                svbr = vpool.tile([128, JC, 2, G], fp16, tag="svbr")
                F0 = vpool.tile([128, 512], bf16, tag="F0")
                F1 = vpool.tile([128, 512], bf16, tag="F1")
                # two persistent [128,2,512] psum tiles, manually ping-
                # ponged for matvec / pre-tile use.  Persistent (not pool-
                # rotated) because matvec matmuls write only partitions
                # {32s}; full-tile reads of rotated tiles trip the race
                # detector on the never-written rows.  memset once below.
                pssA = ps_mv.tile([128, 2, 512], fp32, tag="pssA")
                pssB = ps_mv.tile([128, 2, 512], fp32, tag="pssB")
                nc.vector.memset(pssA[:], 0.0)
                nc.vector.memset(pssB[:], 0.0)
                mv_state = [0]

                def mv_tile():
                    t = pssA if mv_state[0] == 0 else pssB
                    mv_state[0] ^= 1
                    return t
                stg16 = vpool.tile([G, D], fp16, tag="stg16")
                stg32 = vpool.tile([G, D], fp32, tag="stg32")
                stgA = vpool.tile([G, D], fp32, tag="stgA")

                Ms, MTs, ybanks, sbanks = [], [], [], []
                for s in range(G):
                    Ms.append(mpool.tile([128, JC, D], bf16, tag=f"M{s}",
                                         name=f"M{s}"))
                    MTs.append(mpool.tile([128, JC, D], bf16, tag=f"MT{s}",
                                          name=f"MT{s}"))
                    ybanks.append(wrow.tile([37, D], fp16, tag=f"yb{s}",
                                            name=f"yb{s}"))
                    sbanks.append(wrow.tile([37, D], fp16, tag=f"sb{s}",
                                            name=f"sb{s}"))

                def col(jc, s):
                    return slice(jc * G + s, jc * G + s + 1)

                def scol(s):
                    return slice(s, NV, G)

                def setup_splits(c0, srcrow, hi_rows, lo_rows, banks):
                    """fp16 hi/lo split of RT2G*srcrow[c0:c0+G] -> per-
                    sample bank rows via DMA (staged at partition 0 since
                    engine ops need 32-aligned partition starts)."""
                    nc.sync.dma_start(stgA[:], srcrow[c0:c0 + G, :])
                    nc.vector.tensor_scalar_mul(stgA[:], stgA[:], RT2G)
                    nc.vector.tensor_copy(stg16[:], stgA[:])         # hi
                    nc.vector.tensor_copy(stg32[:], stg16[:])
                    nc.vector.tensor_sub(stg32[:], stgA[:], stg32[:])
                    for s in range(G):
                        for r in hi_rows:
                            nc.sync.dma_start(banks[s][r:r + 1, :],
                                              stg16[s:s + 1, :])
                    nc.vector.tensor_copy(stg16[:], stg32[:])        # lo
                    for s in range(G):
                        for r in lo_rows:
                            nc.sync.dma_start(banks[s][r:r + 1, :],
                                              stg16[s:s + 1, :])

                def row_from_cols(svt, s, dst_bank, r0):
                    """transpose-trick: fp16 hi/lo col pairs svt[:, cc, :, s]
                    -> rows r0, r0+1 of dst_bank via K=2 matmuls + copies."""
                    for h in range(2):
                        psr = ps_sm.tile([128, 512], fp32, tag="sm")
                        for c in range(4):
                            cc = h * 4 + c
                            nc.tensor.matmul(
                                psr[0:2, c * 128:(c + 1) * 128],
                                svt[:, cc, :, s], identh[:],
                                start=True, stop=True)
                        if h == 0:
                            nc.scalar.activation(
                                dst_bank[r0:r0 + 2, 0:512],
                                psr[0:2, :], AF.Copy)
                        else:
                            nc.vector.tensor_copy(
                                dst_bank[r0:r0 + 2, 512:1024],
                                psr[0:2, :])

                def hilo_cols(src_f32, s, dst):
                    """fp16 hi/lo split of src_f32[:, scol(s)] into
                    dst[:, :, 0:2, s]"""
                    nc.vector.tensor_copy(dst[:, :, 0, s], src_f32[:, scol(s)])
                    nc.vector.tensor_copy(SCR[:, scol(s)], dst[:, :, 0, s])
                    nc.vector.tensor_sub(SCR[:, scol(s)], src_f32[:, scol(s)],
                                         SCR[:, scol(s)])
                    nc.vector.tensor_copy(dst[:, :, 1, s], SCR[:, scol(s)])

                def rowterm_update():
                    """sbank rows 0,1 <- fp16 hi/lo of (S2-BC) per sample"""
                    nc.vector.tensor_sub(AUX[:], S2[:], BC[:])
                    for s in range(G):
                        hilo_cols(AUX, s, svhp)
                        row_from_cols(svhp, s, sbanks[s], 0)

                def brrow_update():
                    """ybank rows 32,33 <- fp16 hi/lo of BR per sample;
                    also NAUX = BC - S2 (direct-MT exp bias)."""
                    nc.vector.tensor_sub(NAUX[:], BC[:], S2[:])
                    for s in range(G):
                        hilo_cols(BR, s, svbr)
                        row_from_cols(svbr, s, ybanks[s], 32)

                def pre_mm(ps, s, jc, h):
                    nc.tensor.matmul(
                        ps[:, h, :], ybanks[s][0:5, jc * 128:(jc + 1) * 128],
                        sbanks[s][0:5, h * 512:(h + 1) * 512],
                        start=True, stop=True)

                def preT_mm(ps, s, kc, h):
                    nc.tensor.matmul(
                        ps[:, h, :],
                        sbanks[s][32:37, kc * 128:(kc + 1) * 128],
                        ybanks[s][32:37, h * 512:(h + 1) * 512],
                        start=True, stop=True)

                def mbuild_M():
                    """M = exp(pre + BR)"""
                    for s in range(G):
                        for jc in range(JC):
                            ps = mv_tile()
                            pre_mm(ps, s, jc, 0)
                            pre_mm(ps, s, jc, 1)
                            nc.scalar.activation(
                                Ms[s][:, jc, :], ps[:],
                                AF.Exp, bias=BR[:, col(jc, s)])

                def mbuild_MT():
                    """MT hybrid direct/transpose"""
                    for s in range(G):
                        for kc in range(JC):
                            if kc < hybrid_kc:
                                ps = mv_tile()
                                preT_mm(ps, s, kc, 0)
                                preT_mm(ps, s, kc, 1)
                                nc.scalar.activation(
                                    MTs[s][:, kc, :], ps[:],
                                    AF.Exp, bias=NAUX[:, col(kc, s)])
                            else:
                                for hj in range(2):
                                    ptr = ps_tr.tile([128, 512], bf16,
                                                     tag="tr")
                                    for q in range(4):
                                        jc = hj * 4 + q
                                        nc.tensor.transpose(
                                            ptr[:, q * 128:(q + 1) * 128],
                                            Ms[s][:, jc,
                                                  kc * 128:(kc + 1) * 128],
                                            identb[:])
                                    nc.vector.tensor_copy(
                                        MTs[s][:, kc,
                                               hj * 512:(hj + 1) * 512],
                                        ptr[:])

                def side(use_mt, invec, uf_out, svb_out):
                    """uf_out None => skip f32 copy (only needed for the
                    absorb at segment end)"""
                    """one matvec side for the whole cohort: 8 interleaved
                    accumulation chains on 4 PE col groups."""
                    pss = mv_tile()
                    for kc in range(JC):
                        for h in range(2):
                            for s in range(G):
                                mat = MTs[s] if use_mt else Ms[s]
                                nc.tensor.matmul(
                                    pss[32 * s:32 * s + 1, h, :],
                                    invec[:, col(kc, s)],
                                    mat[:, kc, h * 512:(h + 1) * 512],
                                    start=(kc == 0), stop=(kc == JC - 1),
                                    tile_position=(0, 32 * s))
                    # full-tile copies (engines need unit partition step);
                    # sample rows sit at partitions 32s inside F0/F1
                    nc.scalar.activation(F0[:], pss[:, 0, :], AF.Copy)
                    nc.vector.tensor_copy(F1[:], pss[:, 1, :])
                    # row->column: psc[:, c*G+s] = F_half[32s, c*128:...]
                    # = (F-chunk)^T @ e_{32s}; strided-identity rhs gives
                    # all 4 samples in one N=4 matmul per chunk
                    psc = ps_sm.tile([128, 512], fp32, tag="sm")
                    for c in range(4):
                        nc.tensor.matmul(
                            psc[:, (c * G):(c * G + G)],
                            F0[:, c * 128:(c + 1) * 128],
                            identb[:, 0:128:32], start=True, stop=True)
                    for c in range(4):
                        nc.tensor.matmul(
                            psc[:, ((c + 4) * G):((c + 4) * G + G)],
                            F1[:, c * 128:(c + 1) * 128],
                            identb[:, 0:128:32], start=True, stop=True)
                    half = NV // 2
                    with nc.allow_low_precision(
                            reason="recip rounds to bf16 on write; same "
                                   "values as f32-recip-then-cast"):
                        nc.vector.reciprocal(svb_out[:, 0:half],
                                             psc[:, 0:half])
                        nc.vector.reciprocal(svb_out[:, half:NV],
                                             psc[:, half:NV])
                    if uf_out is not None:
                        nc.vector.reciprocal(uf_out[:], psc[:, 0:NV])

                LN2x34 = float(34.0 * np.log(2.0))

                def absorb(vec_f32, bias):
                    # two-range ln: act-engine Ln accepts |x| <= 2^64 and
                    # normals only; u spans ~[1e-29, 1e29].  ln(u) =
                    # Ln(u*2^-34)+34ln2 for u>=1, Ln(u*2^34)-34ln2 for u<=1.
                    nc.vector.tensor_scalar_max(SCR[:], vec_f32[:], 1.0)
                    nc.scalar.activation(LNT[:], SCR[:], AF.Ln,
                                         scale=2.0 ** -34)
                    nc.vector.tensor_scalar_add(LNT[:], LNT[:], LN2x34)
                    nc.vector.tensor_scalar_min(SCR[:], vec_f32[:], 1.0)
                    nc.scalar.activation(SCR[:], SCR[:], AF.Ln,
                                         scale=2.0 ** 34)
                    nc.vector.tensor_scalar_sub(SCR[:], SCR[:], LN2x34)
                    nc.vector.tensor_scalar(MSK[:], vec_f32[:], 1.0, None,
                                            op0=ALU.is_ge)
                    nc.vector.select(SCR[:], MSK[:], LNT[:], SCR[:])
                    nc.vector.tensor_add(bias[:], bias[:], SCR[:])

                def seg_body(sl):
                    rowterm_update()
                    brrow_update()
                    mbuild_M()
                    # iteration 1's u-side is u1 = 1/(M @ 1) = 1/rowsum(M):
                    # DVE reduces over M rows, no PE and no MT needed -- so
                    # the whole MT build slides off the critical path into
                    # iteration 1 (v-side streams M, not MT).
                    mv_tile()                      # pool-parity burns so the
                    ps_sm.tile([128, 512], fp32,   # For_i body keeps an even
                               tag="sm", name="smburn")  # rotation count
                    for s in range(G):
                        for jc in range(JC):
                            nc.vector.tensor_reduce(
                                out=SCR[:, col(jc, s)],
                                in_=Ms[s][:, jc, :],
                                op=ALU.add, axis=AX.X)
                    half = NV // 2
                    with nc.allow_low_precision(
                            reason="recip rounds to bf16 on write"):
                        nc.vector.reciprocal(svb_u[:, 0:half],
                                             SCR[:, 0:half])
                        nc.vector.reciprocal(svb_u[:, half:NV],
                                             SCR[:, half:NV])
                    mbuild_MT()
                    side(False, svb_u, None, svb_v)
                    for t in range(1, sl - 1):
                        side(True, svb_v, None, svb_u)
                        side(False, svb_u, None, svb_v)
                    side(True, svb_v, UF, svb_u)
                    side(False, svb_u, VF, svb_v)
                    absorb(UF, BR)
                    absorb(VF, BC)

                for c0 in range(0, ns, G):
                    # ---- per-sample setup ----
                    setup_splits(c0, y_sb, hi_rows=(2, 3, 34, 35),
                                 lo_rows=(4, 36), banks=ybanks)
                    setup_splits(c0, src_sb, hi_rows=(2, 4, 34, 36),
                                 lo_rows=(3, 35), banks=sbanks)
                    for s in range(G):
                        nc.gpsimd.memset(ybanks[s][0:2, :], -1.0)
                        nc.gpsimd.memset(sbanks[s][32:34, :], 1.0)
                        nc.scalar.activation(S2[:, scol(s)],
                                             srccol[:, c0 + s, :],
                                             AF.Square, scale=RTG)
                    nc.gpsimd.memset(BC[:], 0.0)

                    # ---- init pass: BR = -max_k(pre with BC=0) ----
                    rowterm_update()
                    for s in range(G):
                        for jc in range(JC):
                            ps = mv_tile()
                            pre_mm(ps, s, jc, 0)
                            pre_mm(ps, s, jc, 1)
                            nc.vector.tensor_reduce(
                                out=SCR[:, col(jc, s)], in_=ps[:, 0, :],
                                op=ALU.max, axis=AX.X)
                            nc.vector.tensor_reduce(
                                out=LNT[:, col(jc, s)], in_=ps[:, 1, :],
                                op=ALU.max, axis=AX.X)
                    nc.vector.tensor_max(SCR[:], SCR[:], LNT[:])
                    nc.vector.tensor_scalar_mul(BR[:], SCR[:], -1.0)

                    # ---- segments ----
                    if seg_lens[0] == seg_lens[1] and len(seg_lens) == 3:
                        with tc.For_i(0, 2, 1, hint_engines=(ET.PE,)):
                            seg_body(seg_lens[0])
                        seg_body(seg_lens[2])
                    else:
                        for sl in seg_lens:
                            seg_body(sl)

                    # ---- P accumulation: D*P = exp(pre + BR) --------------
                    rowterm_update()
                    for s in range(G):
                        for jc in range(JC):
                            ps = mv_tile()
                            pre_mm(ps, s, jc, 0)
                            pre_mm(ps, s, jc, 1)
                            nc.scalar.activation(
                                MTs[s][:, jc, :], ps[:],
                                AF.Exp, bias=BR[:, col(jc, s)])
                    for jc in range(JC):
                        for h in range(2):
                            pa = ps_sm.tile([128, 512], fp32, tag="sm")
                            for s in range(G):
                                nc.tensor.matmul(
                                    pa[:], identb[:],
                                    MTs[s][:, jc, h * 512:(h + 1) * 512],
                                    start=(s == 0), stop=(s == G - 1))
                            nc.vector.tensor_add(
                                pacc[:, jc, h * 512:(h + 1) * 512],
                                pacc[:, jc, h * 512:(h + 1) * 512],
                                pa[:])

            # ------------- phase 3: AllReduce + finale ----------------------
            pacc_b = dpool.tile([D, D], fp32)
            pall_b = dpool.tile(
                [D, D], fp32,
                addr_space="Shared" if n_cores > 4 else "Local")
            nc.sync.dma_start(
                pacc_b[:].rearrange("(jc p) k -> p jc k", p=128), pacc[:])
            if skip_collective:
                nc.sync.dma_start(pall_b[:], pacc_b[:])
            else:
                import concourse.mybir as mybir2
                nc.gpsimd.collective_compute(
                    "AllReduce", mybir2.AluOpType.add,
                    replica_groups=[list(range(n_cores))],
                    ins=[pacc_b.opt()], outs=[pall_b.opt()],
                )
            with (
                tc.tile_pool(name="fin", bufs=1) as fpool,
                tc.tile_pool(name="ph3ps", bufs=2, space="PSUM") as ph3ps,
            ):
                ot = fpool.tile([128, JC, D], fp32)
                nc.sync.dma_start(
                    ot[:], pall_b[:].rearrange("(jc p) k -> p jc k", p=128))
                dl = fpool.tile([128, JC, D], fp32)
                nc.sync.dma_start(
                    dl[:],
                    delta_d.ap().rearrange("(jc p) k -> p jc k", p=128))
                nc.vector.tensor_scalar_mul(ot[:], ot[:], SCALE / N_GLOB)
                nc.vector.tensor_add(ot[:], ot[:], dl[:])
                out_sb = fpool.tile([ns, D], fp32)
                for h in range(2):
                    pso = ph3ps.tile([128, 512], fp32, tag="p3")
                    for jc in range(JC):
                        nc.tensor.matmul(
                            pso[:ns], srccol[:, :, jc],
                            ot[:, jc, h * 512:(h + 1) * 512],
                            start=(jc == 0), stop=(jc == JC - 1))
                    nc.scalar.activation(
                        out_sb[:, h * 512:(h + 1) * 512], pso[:ns], AF.Copy)
                nc.sync.dma_start(out_d.ap()[:], out_sb[:])

    nc.compile()
    return nc


def kernel(**inputs):
    X = np.ascontiguousarray(inputs["X"], np.float32)
    Y = np.ascontiguousarray(inputs["Y"], np.float32)
    W = np.ascontiguousarray(inputs["W"], np.float32)
    b = np.ascontiguousarray(inputs["b"], np.float32).reshape(1, D)
    delta = np.ascontiguousarray(inputs["delta_ot"], np.float32)

    from concourse import bass_utils

    if "nc" not in _cache:
        _cache["nc"] = build()
    nc = _cache["nc"]

    in_maps = []
    for c in range(N_CORES):
        sl = slice(c * NS, (c + 1) * NS)
        in_maps.append({
            "x": X[sl], "y": Y[sl], "w": W, "bvec": b, "delta": delta,
        })
    res = bass_utils.run_bass_kernel_spmd(
        nc, in_maps, core_ids=list(range(N_CORES)))
    out = np.concatenate([res.results[c]["out"] for c in range(N_CORES)],
                         axis=0)
    return out.astype(np.float32)


if __name__ == "__main__":
    import reference
    ins = reference.setup_inputs()
    ins = {k: np.asarray(v) for k, v in ins.items()}
    got = kernel(**ins)
    print("out", got.shape, got.dtype)
